# revision 1
# baseline (speedup 1.0000x reference)
"""GCN layer (SpMM): out[r] = sum_{e: row(e)=r} val[e] * embeds[col(e)]
for N=100000 nodes, d=128, E=3200000 edges, distributed over 8 NeuronCores.

Sharding: 1D row partition — core k owns destination rows [k*12500, (k+1)*12500);
the embedding table is replicated. Per core the edges are bucketed by
(128-row output window, 25000-row column chunk); each bucket is padded to a
common slot budget so one SPMD program serves all cores.

Device pipeline per window:
  - 4x dma_gather (one per column chunk, int16 chunk-relative indices) pull
    the 512B embedding rows for up to `budget` edges each into SBUF.
  - per 128-edge subtile, one fused DVE tensor_scalar builds the val-weighted
    one-hot S[e, r] = val[e] * (row_rel[e] == r) from a constant iota tile.
  - TensorE matmuls S^T @ G accumulate the window's [128,128] block in PSUM.
  - PSUM -> SBUF -> DRAM.
"""

import sys

import numpy as np

for _p in ("/opt/trn_rl_repo", "/root/problem"):
    if _p not in sys.path:
        sys.path.insert(0, _p)

N_NODES = 100000
D = 128
N_CORES = 8
B = N_NODES // N_CORES          # 12500 destination rows per core
WIN = 128                       # output window rows (= PSUM partition dim)
NW = (B + WIN - 1) // WIN       # 98 windows per core
B_PAD = NW * WIN                # 12544 padded rows per core
NCH = 4                         # column chunks (int16 index range)
CH = N_NODES // NCH             # 25000 rows per chunk

_cache = {}


def _build(budget, repeat=1):
    """Build + schedule the SPMD bass program for a per-(window,chunk) slot
    budget (multiple of 128). Returns the compiled Bacc module.

    repeat > 1 wraps the compute body in an on-device For_i loop — used only
    by the perf harness to amortize dispatch overhead when measuring."""
    import contextlib

    import concourse.mybir as mybir
    import concourse.tile as tile
    from concourse import bacc

    nsub_ch = budget // 128          # subtiles per chunk segment
    nsub = NCH * nsub_ch             # subtiles per window
    idx_cols = budget // 16          # idx16 columns per (window, chunk)

    nc = bacc.Bacc("TRN2", target_bir_lowering=False, debug=False,
                   num_devices=N_CORES, num_swdge_queues=4)
    embeds = nc.dram_tensor("embeds", [N_NODES, D], mybir.dt.float32,
                            kind="ExternalInput")
    idx16 = nc.dram_tensor("idx16", [128, NW * NCH * idx_cols], mybir.dt.int16,
                           kind="ExternalInput")
    rows_rel = nc.dram_tensor("rows_rel", [128, NW * nsub], mybir.dt.float32,
                              kind="ExternalInput")
    vals = nc.dram_tensor("vals", [128, NW * nsub], mybir.dt.float32,
                          kind="ExternalInput")
    out = nc.dram_tensor("out", [B_PAD, D], mybir.dt.float32,
                         kind="ExternalOutput")

    with tile.TileContext(nc) as tc:
        with (
            tc.tile_pool(name="const", bufs=1) as const_pool,
            tc.tile_pool(name="gather", bufs=3) as g_pool,
            tc.tile_pool(name="s", bufs=8) as s_pool,
            tc.tile_pool(name="o", bufs=2) as o_pool,
            tc.tile_pool(name="psum", bufs=6, space="PSUM") as psum_pool,
        ):
            iota_i = const_pool.tile([128, 128], mybir.dt.int32)
            nc.gpsimd.iota(iota_i[:], pattern=[[1, 128]], base=0,
                           channel_multiplier=0)
            iota_f = const_pool.tile([128, 128], mybir.dt.float32)
            nc.vector.tensor_copy(out=iota_f[:], in_=iota_i[:])

            idx_all = const_pool.tile([128, NW * NCH * idx_cols], mybir.dt.int16)
            nc.sync.dma_start(out=idx_all[:], in_=idx16[:])
            rows_all = const_pool.tile([128, NW * nsub], mybir.dt.float32)
            nc.sync.dma_start(out=rows_all[:], in_=rows_rel[:])
            vals_all = const_pool.tile([128, NW * nsub], mybir.dt.float32)
            nc.sync.dma_start(out=vals_all[:], in_=vals[:])

            loop = tc.For_i(0, repeat, 1) if repeat > 1 else contextlib.nullcontext()
            with loop:
                for w in range(NW):
                    G = g_pool.tile([128, nsub * 128], mybir.dt.float32)
                    for c in range(NCH):
                        seg = G[:, c * budget : (c + 1) * budget]
                        ioff = (w * NCH + c) * idx_cols
                        nc.gpsimd.dma_gather(
                            out_ap=seg.rearrange("p (j d) -> p j d", d=128),
                            in_ap=embeds[c * CH : (c + 1) * CH, :],
                            idxs_ap=idx_all[:, ioff : ioff + idx_cols],
                            num_idxs=budget,
                            num_idxs_reg=budget,
                            elem_size=D,
                            single_packet=False,
                            queue_num=c,
                        )
                    acc = psum_pool.tile([128, 128], mybir.dt.float32, space="PSUM")
                    for j in range(nsub):
                        col = w * nsub + j
                        S = s_pool.tile([128, 128], mybir.dt.float32, tag="S")
                        nc.vector.tensor_scalar(
                            out=S[:],
                            in0=iota_f[:],
                            scalar1=rows_all[:, col : col + 1],
                            scalar2=vals_all[:, col : col + 1],
                            op0=mybir.AluOpType.is_equal,
                            op1=mybir.AluOpType.mult,
                        )
                        nc.tensor.matmul(
                            out=acc[:],
                            lhsT=S[:],
                            rhs=G[:, j * 128 : (j + 1) * 128],
                            start=(j == 0),
                            stop=(j == nsub - 1),
                        )
                    o = o_pool.tile([128, 128], mybir.dt.float32)
                    nc.scalar.copy(out=o[:], in_=acc[:])
                    nc.sync.dma_start(out=out[w * 128 : (w + 1) * 128, :], in_=o[:])

    nc.compile()
    return nc


def _prep(edge_index, edge_vals):
    """Bucket + pad edges; returns (budget, per-core input dicts)."""
    rows = np.asarray(edge_index[0], dtype=np.int64)
    cols = np.asarray(edge_index[1], dtype=np.int64)
    vals = np.asarray(edge_vals, dtype=np.float32)
    E = rows.shape[0]

    core = rows // B
    row_local = rows - core * B
    w = row_local // WIN
    row_rel = (row_local - w * WIN).astype(np.float32)
    ch = cols // CH
    col_rel = (cols - ch * CH).astype(np.int16)

    bucket = ((core * NW + w) * NCH + ch).astype(np.int64)
    n_buckets = N_CORES * NW * NCH
    counts = np.bincount(bucket, minlength=n_buckets)
    budget = int(-(-counts.max() // 128) * 128)

    order = np.argsort(bucket, kind="stable")
    starts = np.zeros(n_buckets, dtype=np.int64)
    np.cumsum(counts[:-1], out=starts[1:])
    pos = np.arange(E, dtype=np.int64) - starts[bucket[order]]

    bo = bucket[order]
    slot = bo * budget + pos            # global slot id across all cores

    n_slots = n_buckets * budget
    idx_lin = np.zeros(n_slots, dtype=np.int16)
    rows_lin = np.zeros(n_slots, dtype=np.float32)
    vals_lin = np.zeros(n_slots, dtype=np.float32)
    idx_lin[slot] = col_rel[order]
    rows_lin[slot] = row_rel[order]
    vals_lin[slot] = vals[order]

    nsub_ch = budget // 128
    nsub = NCH * nsub_ch
    in_maps = []
    per_core = NW * NCH * budget
    for k in range(N_CORES):
        lin = slice(k * per_core, (k + 1) * per_core)
        # idx16: [NW, NCH, budget] -> per segment [16, budget//16], tiled x8
        a = idx_lin[lin].reshape(NW, NCH, budget // 16, 16)
        idx16 = np.ascontiguousarray(
            a.transpose(3, 0, 1, 2).reshape(16, -1))
        idx16 = np.tile(idx16, (8, 1))
        # rows/vals: [NW, NCH*budget] ; slot i -> (partition i%128, col i//128)
        r = rows_lin[lin].reshape(NW, nsub, 128)
        rows_t = np.ascontiguousarray(r.transpose(2, 0, 1).reshape(128, -1))
        v = vals_lin[lin].reshape(NW, nsub, 128)
        vals_t = np.ascontiguousarray(v.transpose(2, 0, 1).reshape(128, -1))
        in_maps.append({"idx16": idx16, "rows_rel": rows_t, "vals": vals_t})
    return budget, in_maps


def kernel(embeds, edge_index, edge_vals):
    from concourse.bass_utils import run_bass_kernel_spmd

    embeds = np.ascontiguousarray(np.asarray(embeds, dtype=np.float32))
    budget, in_maps = _prep(edge_index, edge_vals)
    for m in in_maps:
        m["embeds"] = embeds

    if budget not in _cache:
        _cache[budget] = _build(budget)
    nc = _cache[budget]

    res = run_bass_kernel_spmd(nc, in_maps, core_ids=list(range(N_CORES)))
    out = np.empty((N_NODES, D), dtype=np.float32)
    for k in range(N_CORES):
        out[k * B : (k + 1) * B] = res.results[k]["out"][:B]
    return out



# revision 5
# speedup vs baseline: 3.4964x; 3.4964x over previous
"""GCN layer (SpMM): out[r] = sum_{e: row(e)=r} val[e] * embeds[col(e)]
for N=100000 nodes, d=128, E=3200000 edges, distributed over 8 NeuronCores.

Sharding: 1D row partition — core k owns destination rows [k*12500, (k+1)*12500);
the embedding table is replicated (converted to bf16 host-side). Per core the
edges are bucketed by (128-row output window, 25000-row column chunk); each
bucket is padded to a common slot budget so one SPMD program serves all cores.

Device pipeline per window-group (W_MERGE windows):
  - 4x dma_gather (one per column chunk, int16 chunk-relative indices) pull
    256B bf16 embedding rows for W_MERGE*budget edges each into SBUF.
  - per 128-edge subtile, one fused DVE tensor_scalar builds the val-weighted
    one-hot S[e, r] = val[e] * (row_rel[e] == r) from a constant iota tile
    (all bf16 — 2x DVE rate, 1 cycle/row PE rate).
  - TensorE matmuls S^T @ G accumulate the window's [128,128] block in PSUM
    (fp32 accumulation).
  - PSUM -> SBUF -> DRAM in fp32.
"""

import sys

import numpy as np

for _p in ("/opt/trn_rl_repo", "/root/problem"):
    if _p not in sys.path:
        sys.path.insert(0, _p)

N_NODES = 100000
D = 128
N_CORES = 8
B = N_NODES // N_CORES          # 12500 destination rows per core
WIN = 128                       # output window rows (= PSUM partition dim)
NW = (B + WIN - 1) // WIN       # 98 windows per core
B_PAD = NW * WIN                # 12544 padded rows per core
NCH = 4                         # column chunks (int16 index range)
CH = N_NODES // NCH             # 25000 rows per chunk
WM = 2                          # windows per gather group
NG = NW // WM                   # 49 groups per core

_cache = {}


def _build(budget, repeat=1, num_devices=N_CORES):
    """Build + schedule the SPMD bass program for a per-(window,chunk) slot
    budget (multiple of 128). Returns the compiled Bacc module.

    repeat > 1 wraps the compute body in an on-device For_i loop — used only
    by the perf harness to amortize dispatch overhead when measuring."""
    import contextlib

    import concourse.mybir as mybir
    import concourse.tile as tile
    from concourse import bacc

    nsub_ch = budget // 128          # subtiles per chunk segment
    nsub = NCH * nsub_ch             # subtiles per window
    idx_cols = budget // 16          # idx16 columns per (window, chunk)

    nc = bacc.Bacc("TRN2", target_bir_lowering=False, debug=False,
                   num_devices=num_devices, num_swdge_queues=4)
    embeds = nc.dram_tensor("embeds", [N_NODES, D], mybir.dt.bfloat16,
                            kind="ExternalInput")
    idx16 = nc.dram_tensor("idx16", [128, NW * NCH * idx_cols], mybir.dt.int16,
                           kind="ExternalInput")
    rows_rel = nc.dram_tensor("rows_rel", [128, NW * nsub], mybir.dt.float32,
                              kind="ExternalInput")
    vals = nc.dram_tensor("vals", [128, NW * nsub], mybir.dt.float32,
                          kind="ExternalInput")
    out = nc.dram_tensor("out", [B_PAD, D], mybir.dt.float32,
                         kind="ExternalOutput")

    with tile.TileContext(nc) as tc:
        with (
            tc.tile_pool(name="const", bufs=1) as const_pool,
            tc.tile_pool(name="gather", bufs=3) as g_pool,
            tc.tile_pool(name="s", bufs=8) as s_pool,
            tc.tile_pool(name="o", bufs=2) as o_pool,
            tc.tile_pool(name="psum", bufs=6, space="PSUM") as psum_pool,
        ):
            iota_i = const_pool.tile([128, 128], mybir.dt.int32)
            nc.gpsimd.iota(iota_i[:], pattern=[[1, 128]], base=0,
                           channel_multiplier=0)
            iota_f32 = const_pool.tile([128, 128], mybir.dt.float32)
            nc.vector.tensor_copy(out=iota_f32[:], in_=iota_i[:])
            iota_f = const_pool.tile([128, 128], mybir.dt.bfloat16)
            nc.vector.tensor_copy(out=iota_f[:], in_=iota_f32[:])

            idx_all = const_pool.tile([128, NW * NCH * idx_cols], mybir.dt.int16)
            nc.sync.dma_start(out=idx_all[:], in_=idx16[:])
            rows_all = const_pool.tile([128, NW * nsub], mybir.dt.float32)
            nc.sync.dma_start(out=rows_all[:], in_=rows_rel[:])
            vals_all = const_pool.tile([128, NW * nsub], mybir.dt.float32)
            nc.sync.dma_start(out=vals_all[:], in_=vals[:])

            loop = tc.For_i(0, repeat, 1) if repeat > 1 else contextlib.nullcontext()
            with loop:
                for g in range(NG):
                    # G layout: [128, NCH * WM * nsub_ch * 128]; chunk c's
                    # gather covers windows g*WM..g*WM+WM-1 contiguously.
                    G = g_pool.tile([128, WM * nsub * 128], mybir.dt.bfloat16)
                    for c in range(NCH):
                        seg = G[:, c * WM * budget: (c + 1) * WM * budget]
                        ioff = (g * NCH + c) * WM * idx_cols
                        nc.gpsimd.dma_gather(
                            out_ap=seg.rearrange("p (j d) -> p j d", d=128),
                            in_ap=embeds[c * CH: (c + 1) * CH, :],
                            idxs_ap=idx_all[:, ioff: ioff + WM * idx_cols],
                            num_idxs=WM * budget,
                            num_idxs_reg=WM * budget,
                            elem_size=D,
                            single_packet=False,
                            queue_num=c,
                        )
                    for wi in range(WM):
                        w = g * WM + wi
                        acc = psum_pool.tile([128, 128], mybir.dt.float32,
                                             space="PSUM")
                        for j in range(nsub):
                            c, jj = divmod(j, nsub_ch)
                            col = w * nsub + j
                            gsub = (c * WM + wi) * nsub_ch + jj
                            S = s_pool.tile([128, 128], mybir.dt.bfloat16,
                                            tag="S")
                            nc.vector.tensor_scalar(
                                out=S[:],
                                in0=iota_f[:],
                                scalar1=rows_all[:, col: col + 1],
                                scalar2=vals_all[:, col: col + 1],
                                op0=mybir.AluOpType.is_equal,
                                op1=mybir.AluOpType.mult,
                            )
                            nc.tensor.matmul(
                                out=acc[:],
                                lhsT=S[:],
                                rhs=G[:, gsub * 128: (gsub + 1) * 128],
                                start=(j == 0),
                                stop=(j == nsub - 1),
                            )
                        o = o_pool.tile([128, 128], mybir.dt.float32)
                        nc.scalar.copy(out=o[:], in_=acc[:])
                        nc.sync.dma_start(out=out[w * 128: (w + 1) * 128, :],
                                          in_=o[:])

    nc.compile()
    return nc


def _prep(edge_index, edge_vals):
    """Bucket + pad edges; returns (budget, per-core input dicts)."""
    rows = np.asarray(edge_index[0], dtype=np.int64)
    cols = np.asarray(edge_index[1], dtype=np.int64)
    vals = np.asarray(edge_vals, dtype=np.float32)
    E = rows.shape[0]

    core = rows // B
    row_local = rows - core * B
    w = row_local // WIN
    row_rel = (row_local - w * WIN).astype(np.float32)
    ch = cols // CH
    col_rel = (cols - ch * CH).astype(np.int16)

    bucket = ((core * NW + w) * NCH + ch).astype(np.int64)
    n_buckets = N_CORES * NW * NCH
    counts = np.bincount(bucket, minlength=n_buckets)
    budget = int(-(-counts.max() // 128) * 128)

    order = np.argsort(bucket, kind="stable")
    starts = np.zeros(n_buckets, dtype=np.int64)
    np.cumsum(counts[:-1], out=starts[1:])
    pos = np.arange(E, dtype=np.int64) - starts[bucket[order]]

    bo = bucket[order]
    slot = bo * budget + pos            # global slot id across all cores

    n_slots = n_buckets * budget
    idx_lin = np.zeros(n_slots, dtype=np.int16)
    rows_lin = np.zeros(n_slots, dtype=np.float32)
    vals_lin = np.zeros(n_slots, dtype=np.float32)
    idx_lin[slot] = col_rel[order]
    rows_lin[slot] = row_rel[order]
    vals_lin[slot] = vals[order]

    nsub_ch = budget // 128
    nsub = NCH * nsub_ch
    in_maps = []
    per_core = NW * NCH * budget
    for k in range(N_CORES):
        lin = slice(k * per_core, (k + 1) * per_core)
        # idx16 gather order: [NG, NCH, WM, budget] — chunk c of group g
        # reads WM windows' idx streams back to back. Per 16-slot block the
        # slots wrap into 16 partitions, tiled x8 across the 128 partitions.
        a = idx_lin[lin].reshape(NG, WM, NCH, budget // 16, 16)
        a = a.transpose(4, 0, 2, 1, 3)      # [16, NG, NCH, WM, budget//16]
        idx16 = np.ascontiguousarray(a.reshape(16, -1))
        idx16 = np.tile(idx16, (8, 1))
        # rows/vals: [NW, NCH*budget] ; slot i -> (partition i%128, col i//128)
        r = rows_lin[lin].reshape(NW, nsub, 128)
        rows_t = np.ascontiguousarray(r.transpose(2, 0, 1).reshape(128, -1))
        v = vals_lin[lin].reshape(NW, nsub, 128)
        vals_t = np.ascontiguousarray(v.transpose(2, 0, 1).reshape(128, -1))
        in_maps.append({"idx16": idx16, "rows_rel": rows_t, "vals": vals_t})
    return budget, in_maps


def kernel(embeds, edge_index, edge_vals):
    import ml_dtypes

    from concourse.bass_utils import run_bass_kernel_spmd

    embeds = np.ascontiguousarray(
        np.asarray(embeds, dtype=np.float32).astype(ml_dtypes.bfloat16))
    budget, in_maps = _prep(edge_index, edge_vals)
    for m in in_maps:
        m["embeds"] = embeds

    if budget not in _cache:
        _cache[budget] = _build(budget)
    nc = _cache[budget]

    res = run_bass_kernel_spmd(nc, in_maps, core_ids=list(range(N_CORES)))
    out = np.empty((N_NODES, D), dtype=np.float32)
    for k in range(N_CORES):
        out[k * B : (k + 1) * B] = res.results[k]["out"][:B]
    return out


# revision 12
# speedup vs baseline: 5.8751x; 1.6803x over previous
"""GCN layer (SpMM): out[r] = sum_{e: row(e)=r} val[e] * embeds[col(e)]
for N=100000 nodes, d=128, E=3200000 edges, distributed over 8 NeuronCores.

Sharding: 1D row partition — core k owns destination rows [k*12500, (k+1)*12500);
the embedding table is replicated (converted to bf16 host-side). Per core the
edges are bucketed by (128-row output window, 25000-row column chunk); each
bucket is padded to a common slot budget so one SPMD program serves all cores.

Device pipeline per window-group (WM windows):
  - 4x dma_gather (one per column chunk, int16 chunk-relative indices) pull
    256B bf16 embedding rows for WM*budget edges each into SBUF.
  - one ONEHOT_VAL_ANT custom DVE instruction per window builds the whole
    window's val-weighted one-hot block S[e, j*128+r] = val[e,j] *
    (row_rel[e,j] + 128j == j*128+r) from stride-0 broadcast row/val streams.
  - TensorE matmuls S^T @ G accumulate the window's [128,128] block in PSUM
    (fp32 accumulation), one 128-edge subtile at a time.
  - PSUM -> SBUF -> DRAM in fp32.
"""

import os
import sys

import numpy as np

for _p in ("/opt/trn_rl_repo", "/root/problem"):
    if _p not in sys.path:
        sys.path.insert(0, _p)

N_NODES = 100000
D = 128
N_CORES = 8
B = N_NODES // N_CORES          # 12500 destination rows per core
WIN = 128                       # output window rows (= PSUM partition dim)
NW = (B + WIN - 1) // WIN       # 98 windows per core
B_PAD = NW * WIN                # 12544 padded rows per core
NCH = 4                         # column chunks (int16 index range)
CH = N_NODES // NCH             # 25000 rows per chunk
WM_DEFAULT = 1                  # windows per gather group

_cache = {}
_onehot_op = None


def _get_onehot_op():
    """Register (once) and return the ONEHOT_VAL_ANT custom DVE op:
    out[p, k] = (in0[p, k] == k) * in1[p, k], with in0/in1 usually stride-0
    broadcast APs so one instruction builds a whole window of val-weighted
    one-hot subtiles. The uops sha is computed at registration, self-pinning
    the table bytes."""
    global _onehot_op
    if _onehot_op is not None:
        return _onehot_op
    import concourse.dve_ops as dve_ops
    from concourse.dve_spec import Spec, Src0, Src1, Idx, eq, lower, _has_src1
    from concourse.dve_uop import DveOpSpec

    NAME = "ONEHOT_VAL_ANT"

    def _ref(in0, in1, c0, c1, c2):
        P = in0.shape[0]
        x = np.asarray(in0, np.float32).reshape(P, -1)
        v = np.asarray(in1, np.float32).reshape(P, -1)
        idx = np.arange(x.shape[1], dtype=np.float32)[None, :]
        return (x == idx).astype(np.float32) * v

    if NAME in dve_ops._SUB_OPCODE_FOR_NAME:
        _onehot_op = next(o for o in dve_ops.OPS if o.name == NAME)
        return _onehot_op
    spec = Spec(body=eq(Idx, Src0) * Src1, reference=_ref)
    row = max(dve_ops._SUB_OPCODE_FOR_NAME.values()) + 1
    assert row < 0x20
    shas = {v: DveOpSpec(name=NAME, opcode=row, uops=lower(spec, ver=v),
                         rd1_en=_has_src1(spec)).sha(v) for v in ("v3", "v4")}
    op = dve_ops.DveOp(NAME, spec, subdim=False, uops_sha=shas)
    dve_ops.OPS.append(op)
    dve_ops.CUSTOM_DVE_SPECS[NAME] = spec
    dve_ops._SUB_OPCODE_FOR_NAME[NAME] = row
    _onehot_op = op
    return op


def _build(budget, repeat=1, num_devices=N_CORES, wm=WM_DEFAULT, skip=False,
           gbufs=3, sbufs=8, pbufs=6, ppad=512, queues=4, sgran=4, sp=False,
           do_gather=True, do_dve=True, do_pe=True):
    """Build + schedule the SPMD bass program for a per-(window,chunk) slot
    budget (multiple of 128). Returns the compiled Bacc module.

    repeat > 1 wraps the compute body in an on-device For_i loop — used only
    by the perf harness to amortize dispatch overhead when measuring.
    The do_* switches build partial variants for bottleneck experiments."""
    import contextlib

    import concourse.mybir as mybir
    import concourse.tile as tile
    from concourse import bacc

    WM, NG = wm, NW // wm
    assert not (skip and wm != 1), "pad-skip needs trailing -1s: wm must be 1"
    nsub_ch = budget // 128          # subtiles per chunk segment
    nsub = NCH * nsub_ch             # subtiles per window
    idx_cols = budget // 16          # idx16 columns per (window, chunk)

    onehot = _get_onehot_op()
    nc = bacc.Bacc("TRN2", target_bir_lowering=False, debug=False,
                   num_devices=num_devices, num_swdge_queues=queues)
    embeds = nc.dram_tensor("embeds", [N_NODES, D], mybir.dt.bfloat16,
                            kind="ExternalInput")
    idx16 = nc.dram_tensor("idx16", [128, NW * NCH * idx_cols], mybir.dt.int16,
                           kind="ExternalInput")
    rows_rel = nc.dram_tensor("rows_rel", [128, NW * nsub], mybir.dt.float32,
                              kind="ExternalInput")
    vals = nc.dram_tensor("vals", [128, NW * nsub], mybir.dt.float32,
                          kind="ExternalInput")
    counts = None
    if skip:
        counts = nc.dram_tensor("counts", [1, NW * NCH], mybir.dt.int32,
                                kind="ExternalInput")
    out = nc.dram_tensor("out", [B_PAD, D], mybir.dt.float32,
                         kind="ExternalOutput")

    with tile.TileContext(nc) as tc:
        with (
            tc.tile_pool(name="const", bufs=1) as const_pool,
            tc.tile_pool(name="gather", bufs=gbufs) as g_pool,
            tc.tile_pool(name="s", bufs=sbufs) as s_pool,
            tc.tile_pool(name="o", bufs=2) as o_pool,
            tc.tile_pool(name="psum", bufs=pbufs, space="PSUM") as psum_pool,
        ):
            idx_all = const_pool.tile([128, NW * NCH * idx_cols], mybir.dt.int16)
            nc.sync.dma_start(out=idx_all[:], in_=idx16[:])
            rows_all = const_pool.tile([128, NW * nsub], mybir.dt.float32)
            nc.sync.dma_start(out=rows_all[:], in_=rows_rel[:])
            vals_all = const_pool.tile([128, NW * nsub], mybir.dt.float32)
            nc.sync.dma_start(out=vals_all[:], in_=vals[:])
            cnt_reg = None
            if skip:
                cnt_all = const_pool.tile([128, NW * NCH], mybir.dt.int32)
                nc.sync.dma_start(out=cnt_all[0:1, :], in_=counts[:])
                cnt_reg = nc.alloc_register(mybir.EngineType.Pool, "gcnt")
                # pad slots use idx -1 and are skipped by the gather; the
                # skipped G regions are never written, so zero the gather
                # pool buffers once so stale-SBUF NaNs can't leak into PSUM
                if do_gather:
                    for zi in range(gbufs):
                        Gz = g_pool.tile([128, WM * nsub * 128],
                                         mybir.dt.bfloat16, tag="G",
                                         name=f"Gz{zi}")
                        nc.vector.memset(Gz[:], 0.0)

            Gc = Sc = None
            if not do_gather:
                Gc = const_pool.tile([128, WM * nsub * 128], mybir.dt.bfloat16)
                nc.vector.memset(Gc[:], 0.25)
            if not do_dve and do_pe:
                Sc = const_pool.tile([128, nsub * 128], mybir.dt.bfloat16)
                nc.vector.memset(Sc[:], 0.5)

            loop = tc.For_i(0, repeat, 1) if repeat > 1 else contextlib.nullcontext()
            with loop:
                for g in range(NG):
                    if do_gather:
                        # G layout: [128, NCH * WM * nsub_ch * 128]; chunk c's
                        # gather covers windows g*WM..g*WM+WM-1 contiguously.
                        G = g_pool.tile([128, WM * nsub * 128],
                                        mybir.dt.bfloat16, tag="G")
                        for c in range(NCH):
                            seg = G[:, c * WM * budget: (c + 1) * WM * budget]
                            ioff = (g * NCH + c) * WM * idx_cols
                            if skip:
                                nc.gpsimd.reg_load(
                                    cnt_reg,
                                    cnt_all[0:1, g * NCH + c: g * NCH + c + 1])
                            nc.gpsimd.dma_gather(
                                out_ap=seg.rearrange("p (j d) -> p j d", d=128),
                                in_ap=embeds[c * CH: (c + 1) * CH, :],
                                idxs_ap=idx_all[:, ioff: ioff + WM * idx_cols],
                                num_idxs=WM * budget,
                                num_idxs_reg=cnt_reg if skip else WM * budget,
                                elem_size=D,
                                single_packet=sp,
                                queue_num=c % queues,
                            )
                    else:
                        G = Gc
                    for wi in range(WM):
                        w = g * WM + wi
                        if do_dve:
                            # sgran S tiles per window: finer S pieces wake
                            # the PE sooner and keep it denser (HAM p-state);
                            # separate tiles keep the write->read deps exact
                            piece = nsub // sgran
                            Sp = []
                            for si in range(sgran):
                                lo = w * nsub + si * piece
                                St = s_pool.tile([128, piece * 128],
                                                 mybir.dt.bfloat16, tag="S",
                                                 name=f"S_{w}_{si}")
                                nc.vector._custom_dve(
                                    onehot,
                                    out=St[:],
                                    in0=rows_all[:, lo: lo + piece]
                                        .rearrange("p (s o) -> p s o", o=1)
                                        .broadcast_to([128, piece, 128]),
                                    in1=vals_all[:, lo: lo + piece]
                                        .rearrange("p (s o) -> p s o", o=1)
                                        .broadcast_to([128, piece, 128]),
                                )
                                Sp.append(St)
                        o = o_pool.tile([128, 128], mybir.dt.float32)
                        if do_pe:
                            acc = psum_pool.tile(
                                [128, 128], mybir.dt.float32, space="PSUM",
                                padded_shape=[128, ppad] if ppad else None)
                            piece = nsub // sgran
                            for j in range(nsub):
                                c, jj = divmod(j, nsub_ch)
                                gsub = (c * WM + wi) * nsub_ch + jj
                                Sj = (Sp[j // piece][:, (j % piece) * 128:
                                                    (j % piece + 1) * 128]
                                      if do_dve else Sc[:, j * 128: (j + 1) * 128])
                                nc.tensor.matmul(
                                    out=acc[:],
                                    lhsT=Sj,
                                    rhs=G[:, gsub * 128: (gsub + 1) * 128],
                                    start=(j == 0),
                                    stop=(j == nsub - 1),
                                )
                            nc.scalar.copy(out=o[:], in_=acc[:])
                        elif do_dve:
                            nc.scalar.copy(out=o[:], in_=Sp[-1][:, :128])
                        else:
                            nc.scalar.copy(out=o[:], in_=G[:, :128])
                        nc.sync.dma_start(out=out[w * 128: (w + 1) * 128, :],
                                          in_=o[:])

    nc.compile()
    return nc


def _prep(edge_index, edge_vals, wm=WM_DEFAULT, skip=False, sgran=4):
    """Bucket + pad edges; returns (budget, per-core input dicts)."""
    WM, NG = wm, NW // wm
    rows = np.asarray(edge_index[0], dtype=np.int64)
    cols = np.asarray(edge_index[1], dtype=np.int64)
    vals = np.asarray(edge_vals, dtype=np.float32)
    E = rows.shape[0]

    core = rows // B
    row_local = rows - core * B
    w = row_local // WIN
    row_rel = (row_local - w * WIN).astype(np.float32)
    ch = cols // CH
    col_rel = (cols - ch * CH).astype(np.int16)

    bucket = ((core * NW + w) * NCH + ch).astype(np.int64)
    n_buckets = N_CORES * NW * NCH
    counts = np.bincount(bucket, minlength=n_buckets)
    budget = int(-(-counts.max() // 128) * 128)

    order = np.argsort(bucket, kind="stable")
    starts = np.zeros(n_buckets, dtype=np.int64)
    np.cumsum(counts[:-1], out=starts[1:])
    pos = np.arange(E, dtype=np.int64) - starts[bucket[order]]

    bo = bucket[order]
    slot = bo * budget + pos            # global slot id across all cores

    n_slots = n_buckets * budget
    fill = -1 if skip else 0
    idx_lin = np.full(n_slots, fill, dtype=np.int16)
    rows_lin = np.zeros(n_slots, dtype=np.float32)
    vals_lin = np.zeros(n_slots, dtype=np.float32)
    idx_lin[slot] = col_rel[order]
    # rows are stored pre-offset by 128*(subtile within S-piece) so the
    # ONEHOT_VAL_ANT custom op can compare against the element index local
    # to its instruction (sgran instructions per window)
    wpos = slot % ((NCH * budget) // sgran)
    rows_lin[slot] = row_rel[order] + 128.0 * (wpos // 128).astype(np.float32)
    vals_lin[slot] = vals[order]

    nsub_ch = budget // 128
    nsub = NCH * nsub_ch
    in_maps = []
    per_core = NW * NCH * budget
    for k in range(N_CORES):
        lin = slice(k * per_core, (k + 1) * per_core)
        # idx16 gather order: [NG, NCH, WM, budget] — chunk c of group g
        # reads WM windows' idx streams back to back. Per 16-slot block the
        # slots wrap into 16 partitions, tiled x8 across the 128 partitions.
        a = idx_lin[lin].reshape(NG, WM, NCH, budget // 16, 16)
        a = a.transpose(4, 0, 2, 1, 3)      # [16, NG, NCH, WM, budget//16]
        idx16 = np.ascontiguousarray(a.reshape(16, -1))
        idx16 = np.tile(idx16, (8, 1))
        # per-(group, chunk) real-edge counts, in gather emission order
        cwc = counts.reshape(N_CORES, NW, NCH)[k]          # [NW, NCH]
        cg = cwc.reshape(NG, WM, NCH).sum(axis=1)          # [NG, NCH]
        cnts = np.ascontiguousarray(
            cg.transpose(0, 1).reshape(1, -1).astype(np.int32))
        # rows/vals: [NW, NCH*budget] ; slot i -> (partition i%128, col i//128)
        r = rows_lin[lin].reshape(NW, nsub, 128)
        rows_t = np.ascontiguousarray(r.transpose(2, 0, 1).reshape(128, -1))
        v = vals_lin[lin].reshape(NW, nsub, 128)
        vals_t = np.ascontiguousarray(v.transpose(2, 0, 1).reshape(128, -1))
        m = {"idx16": idx16, "rows_rel": rows_t, "vals": vals_t}
        if skip:
            m["counts"] = cnts
        in_maps.append(m)
    return budget, in_maps


def kernel(embeds, edge_index, edge_vals):
    import ml_dtypes

    from concourse.bass_utils import run_bass_kernel_spmd

    embeds = np.ascontiguousarray(
        np.asarray(embeds, dtype=np.float32).astype(ml_dtypes.bfloat16))
    budget, in_maps = _prep(edge_index, edge_vals)
    for m in in_maps:
        m["embeds"] = embeds

    if budget not in _cache:
        _cache[budget] = _build(budget)
    nc = _cache[budget]

    res = run_bass_kernel_spmd(nc, in_maps, core_ids=list(range(N_CORES)))
    out = np.empty((N_NODES, D), dtype=np.float32)
    for k in range(N_CORES):
        out[k * B : (k + 1) * B] = res.results[k]["out"][:B]
    return out
